# revision 3
# baseline (speedup 1.0000x reference)
"""MixedFFN Trainium2 kernel (8 NeuronCores, SPMD) — v3.

Problem: x [8, 2048, 1024]; shared FFN (W1S [2048,1024], W2S [1024,2048])
applied to positions 0..1984 of every batch; per-position FFN
(W1NS [64,1024,2048], W2NS [64,2048,1024]) applied to positions 1984..2048.
gelu is exact (erf). Output [8, 2048, 1024] fp32.

Sharding:
  - Shared part: data-parallel over batch. Core i computes the shared FFN
    for batch i (rows 0..1984; the last row-block streams only 448 cols).
  - Per-position part: sharded over positions. Core i handles positions
    1984+8i .. 1984+8(i+1) for ALL batches (two groups of 4 positions).

v3 changes over the 307us v2 (see kernel_v2_baseline.py):
  - NS work emitted as PURE bursts (8 quadrant groups each) with shared
    MM2 lagged one fc behind MM1, so no earlier-emitted tensor work is
    pending-unready at burst boundaries.  The tile scheduler's cost model
    treats each quadrant matmul as a full 512-cycle serial instruction
    (no 4-way concurrency modeling), so any ready shared matmul used to
    split the 4-instruction quadrant groups, serializing their streams
    (~21us lost).  Pure bursts keep groups adjacent -> 216ns/group.
  - NS weight DMAs ride the (otherwise idle) GpSimd HWDGE queue, so
    pool-slot waits self-pace the prefetch without ever blocking the
    shared-critical Sync queue (v2 lost ~4us to w1t/xt stuck behind NS
    DMAs at rb0/rb1).
  - Shared MM2 restructured: rc0 eager (lagged, dh-split psum halves),
    rc1..3 lazy from an hth buffer holding 3 rc blocks.  This frees PSUM:
    ph+phn share a 4-slot rotation, transposes and pyn a 2-slot one, so
    NS MM1(fb+1) overlaps gelu(fb) and transposes never stall the PE.
  - Tail: last emission is the g1 NS MM2 burst (small YN output), so the
    final 1MB of YS drains under NS matmuls instead of after everything.
  - xt loads split in dc-halves so MM1(fc0) starts after 1.25MB, not 2.25.
"""

import os
import sys

import numpy as np

for _p in ("/opt/trn_rl_repo",):
    if os.path.isdir(_p) and _p not in sys.path:
        sys.path.insert(0, _p)

B, T, D, F, LNS = 8, 2048, 1024, 2048, 64
S = T - LNS  # 1984
NCORES = 8
PPC = LNS // NCORES  # 8 positions per core
NG = 2  # groups of 4 positions
FB = 4  # f-blocks of 512 for NS MM1
RB = 512  # shared row-block
NRB = T // RB  # 4
DC, FC = D // 128, F // 128  # 8, 16 k-chunks

LAST_RESULTS = None  # BassKernelResults of the most recent run (for test.py)

_cached = None


def _build():
    import concourse.tile as tile
    from concourse import bacc
    from concourse import mybir

    f32 = mybir.dt.float32
    f16 = mybir.dt.float16
    GELU = (
        mybir.ActivationFunctionType.Relu
        if os.environ.get("MIXEDFFN_SIM_ACT") == "relu"
        else mybir.ActivationFunctionType.Gelu
    )

    nc = bacc.Bacc("TRN2", target_bir_lowering=False, debug=False, num_devices=NCORES)

    XT = nc.dram_tensor("XT", [NRB, 128, DC, RB], f16, kind="ExternalInput").ap()
    XNS = nc.dram_tensor("XNS", [128, DC, NG, 4, 16], f16, kind="ExternalInput").ap()
    W1T = nc.dram_tensor("W1T", [FC, 128, DC, 128], f16, kind="ExternalInput").ap()
    W2T = nc.dram_tensor("W2T", [F, D], f16, kind="ExternalInput").ap()
    W1N = nc.dram_tensor("W1N", [NG, FB, 4, 128, 4096], f16, kind="ExternalInput").ap()
    W2N = nc.dram_tensor("W2N", [NG, 2, 8, 128, 4096], f16, kind="ExternalInput").ap()
    IDEN = nc.dram_tensor("IDEN", [128, 128], f16, kind="ExternalInput").ap()
    YS = nc.dram_tensor("YS", [S, D], f16, kind="ExternalOutput").ap()
    YN = nc.dram_tensor("YN", [NG, 2, 128, 512], f16, kind="ExternalOutput").ap()

    with tile.TileContext(nc) as tc:
        with (
            tc.tile_pool(name="wres", bufs=1) as wres,
            tc.tile_pool(name="xt", bufs=2) as xtp,
            tc.tile_pool(name="hth", bufs=1) as hthp,
            tc.tile_pool(name="ht01", bufs=2) as ht01p,
            tc.tile_pool(name="ysb", bufs=1) as ysbp,
            tc.tile_pool(name="w1n", bufs=6) as w1np,
            tc.tile_pool(name="w2n", bufs=6) as w2np,
            tc.tile_pool(name="hsb", bufs=2) as hsbp,
            tc.tile_pool(name="hT", bufs=4) as htnsp,
            tc.tile_pool(name="ynsb", bufs=2) as ynsbp,
            tc.tile_pool(name="p512", bufs=4, space="PSUM") as p512,
            tc.tile_pool(name="py", bufs=2, space="PSUM") as pyp,
            tc.tile_pool(name="ptp", bufs=2, space="PSUM") as ptp,
        ):
            # ---- warmup: wake the HAM clock during the DMA preamble ----
            warm = wres.tile([128, 512], f16)
            nc.vector.memset(warm[:], 0.0)
            wps = p512.tile([128, 512], f32, name="warmps", tag="p512")
            for _ in range(20):
                nc.tensor.matmul(
                    wps[:], warm[:, 0:128], warm[:], start=True, stop=True,
                    skip_group_check=True,
                )

            # ---- resident tiles ----
            w1t_sb = wres.tile([128, FC, DC, 128], f16)
            w2t_sb = wres.tile([128, FC, D], f16)
            xns_sb = wres.tile([128, DC, NG, 4, 16], f16)
            nc.scalar.dma_start(out=xns_sb[:], in_=XNS[:])
            ident = wres.tile([128, 128], f16)
            nc.scalar.dma_start(out=ident[:], in_=IDEN[:])

            state = {}

            # ---- shared-part steps ----
            def xt_load(rb, half):
                if half == 0:
                    state[("xt", rb)] = xtp.tile(
                        [128, DC, RB], f16, name=f"xtt{rb}", tag="xtt"
                    )
                xt = state[("xt", rb)]
                nc.sync.dma_start(
                    out=xt[:, 4 * half : 4 * half + 4],
                    in_=XT[rb, :, 4 * half : 4 * half + 4],
                )

            def w1t_load(fc):
                nc.sync.dma_start(out=w1t_sb[:, fc], in_=W1T[fc])

            def w2t_load(fc):
                nc.sync.dma_start(
                    out=w2t_sb[:, fc, :], in_=W2T[fc * 128 : (fc + 1) * 128, :]
                )

            def eager(rb, fc):
                ht01 = state[("ht01", fc)]
                if fc == 0:
                    state["py0"] = pyp.tile(
                        [128, 512], f32, name=f"py{rb}d0", tag="py"
                    )
                    state["py1"] = pyp.tile(
                        [128, 512], f32, name=f"py{rb}d1", tag="py"
                    )
                for dh in range(2):
                    nc.tensor.matmul(
                        state[f"py{dh}"][:],
                        ht01[:],
                        w2t_sb[:, fc, dh * 512 : (dh + 1) * 512],
                        start=(fc == 0),
                        stop=(fc == FC - 1),
                        skip_group_check=True,
                    )

            def fc_step(rb, fc):
                R = 448 if rb == NRB - 1 else 512
                xt = state[("xt", rb)]
                if fc == 0:
                    state["hth"] = hthp.tile(
                        [128, FC, 384], f16, name=f"hth{rb}", tag="hth"
                    )
                ph = p512.tile([128, 512], f32, name=f"ph{rb}_{fc}", tag="p512")
                for dc in range(DC):
                    nc.tensor.matmul(
                        ph[:, 0:R],
                        w1t_sb[:, fc, dc, :],
                        xt[:, dc, 0:R],
                        start=(dc == 0),
                        stop=(dc == DC - 1),
                        skip_group_check=True,
                    )
                ht01 = ht01p.tile([128, 128], f16, name=f"ht01_{rb}_{fc}", tag="ht01")
                state[("ht01", fc)] = ht01
                nc.scalar.activation(ht01[:], ph[:, 0:128], GELU)
                nc.scalar.activation(
                    state["hth"][:, fc, 0 : R - 128], ph[:, 128:R], GELU
                )
                if fc > 0:
                    eager(rb, fc - 1)

            def eager_tail(rb):
                eager(rb, FC - 1)

            def y0_out(rb):
                ysb = ysbp.tile([128, D], f16, name=f"ysb{rb}_0", tag="ysb")
                nc.vector.tensor_copy(ysb[:, 0:512], state["py0"][:])
                nc.vector.tensor_copy(ysb[:, 512:1024], state["py1"][:])
                nc.sync.dma_start(out=YS[rb * RB : rb * RB + 128, :], in_=ysb[:])

            def mm2b(rb, rc, dh):
                hth = state["hth"]
                nrows = min(128, S - (rb * RB + rc * 128))
                py = pyp.tile([128, 512], f32, name=f"pyb{rb}_{rc}_{dh}", tag="py")
                for fc in range(FC):
                    nc.tensor.matmul(
                        py[0:nrows, :],
                        hth[:, fc, (rc - 1) * 128 : (rc - 1) * 128 + nrows],
                        w2t_sb[:, fc, dh * 512 : (dh + 1) * 512],
                        start=(fc == 0),
                        stop=(fc == FC - 1),
                        skip_group_check=True,
                    )
                if dh == 0:
                    state["ysbb"] = ysbp.tile(
                        [128, D], f16, name=f"ysb{rb}_{rc}", tag="ysb"
                    )
                ysb = state["ysbb"]
                nc.vector.tensor_copy(
                    ysb[0:nrows, dh * 512 : (dh + 1) * 512], py[0:nrows, :]
                )
                if dh == 1:
                    row0 = rb * RB + rc * 128
                    nc.sync.dma_start(
                        out=YS[row0 : row0 + nrows, :], in_=ysb[0:nrows, :]
                    )

            # ---- NS steps ----
            def ns_w1dma(g, fb, dcp):
                w1 = w1np.tile(
                    [128, 2, 4, 512], f16, name=f"w1_{g}_{fb}_{dcp}", tag="w1"
                )
                state[("w1", g, fb, dcp)] = w1
                nc.gpsimd.dma_start(out=w1[:], in_=W1N[g, fb, dcp])

            def ns_w2dma(g, dh, fcp):
                w2 = w2np.tile(
                    [128, 2, 4, 512], f16, name=f"w2_{g}_{dh}_{fcp}", tag="w2"
                )
                state[("w2", g, dh, fcp)] = w2
                nc.gpsimd.dma_start(out=w2[:], in_=W2N[g, dh, fcp])

            def ns_m1(g, fb):
                phn = p512.tile([128, 512], f32, name=f"phn{g}_{fb}", tag="p512")
                for dc in range(DC):
                    w1 = state[("w1", g, fb, dc // 2)]
                    for j in range(4):
                        nc.tensor.matmul(
                            phn[32 * j : 32 * j + B, :],
                            xns_sb[:, dc, g, j, 0:B],
                            w1[:, dc % 2, j, :],
                            start=(dc == 0),
                            stop=(dc == DC - 1),
                            skip_group_check=True,
                            tile_position=(0, 32 * j),
                        )
                hsb = hsbp.tile([128, 512], f16, name=f"hsb{g}_{fb}", tag="hsb")
                state[("hsb", g, fb)] = hsb
                nc.scalar.activation(hsb[:], phn[:], GELU)

            def ns_tr(g, fb):
                hsb = state[("hsb", g, fb)]
                pt = ptp.tile([128, 4, 128], f16, name=f"pt{g}_{fb}", tag="ptp")
                for k in range(4):
                    nc.tensor.transpose(
                        pt[:, k, :], hsb[:, k * 128 : (k + 1) * 128], ident[:]
                    )
                hT = htnsp.tile([128, 4, 128], f16, name=f"hT{g}_{fb}", tag="hT")
                state[("hT", g, fb)] = hT
                nc.vector.tensor_copy(hT[:], pt[:])

            def ns_m2(g, dh, h):
                if h == 0:
                    state[("pyn", dh)] = ptp.tile(
                        [128, 512], f32, name=f"pyn{g}_{dh}", tag="ptp"
                    )
                pyn = state[("pyn", dh)]
                for fc in range(8 * h, 8 * h + 8):
                    w2 = state[("w2", g, dh, fc // 2)]
                    hT = state[("hT", g, fc // 4)]
                    for j in range(4):
                        nc.tensor.matmul(
                            pyn[32 * j : 32 * j + B, :],
                            hT[:, fc % 4, 32 * j : 32 * j + B],
                            w2[:, fc % 2, j, :],
                            start=(fc == 0),
                            stop=(fc == FC - 1),
                            skip_group_check=True,
                            tile_position=(0, 32 * j),
                        )
                if h == 1:
                    ynsb = ynsbp.tile(
                        [128, 512], f16, name=f"ynsb{g}_{dh}", tag="ynsb"
                    )
                    nc.vector.tensor_copy(ynsb[:], pyn[:])
                    nc.sync.dma_start(out=YN[g, dh], in_=ynsb[:])

            # ---- NS unit schedule ----
            # Per group: 4 MM1 bursts, 4 transpose mini-steps (pipelined one
            # fb behind), 4 MM2 half-bursts.  Units run in this fixed order;
            # each unit's weight DMAs are emitted ~2 big-bursts ahead on the
            # gpsimd queue, self-paced by the 6-deep tile pools.
            def unit_order(g):
                return [
                    ("m1", g, 0), ("m1", g, 1), ("tr", g, 0), ("m1", g, 2),
                    ("tr", g, 1), ("m1", g, 3), ("tr", g, 2), ("tr", g, 3),
                    ("m2", g, 0, 0), ("m2", g, 0, 1),
                    ("m2", g, 1, 0), ("m2", g, 1, 1),
                ]

            ns_units = unit_order(0) + unit_order(1)
            dma_groups = []
            for u in ns_units:
                if u[0] == "m1":
                    _, g, fb = u
                    dma_groups.append([(g, "w1", fb, dcp) for dcp in range(4)])
                elif u[0] == "m2":
                    _, g, dh, h = u
                    dma_groups.append(
                        [(g, "w2", dh, fcp) for fcp in range(4 * h, 4 * h + 4)]
                    )

            def emit_dma_group(k):
                if k >= len(dma_groups):
                    return
                for it in dma_groups[k]:
                    if it[1] == "w1":
                        ns_w1dma(it[0], it[2], it[3])
                    else:
                        ns_w2dma(it[0], it[2], it[3])

            big_idx = [0]  # count of big bursts emitted

            def emit_unit(u):
                if u[0] == "m1":
                    ns_m1(u[1], u[2])
                elif u[0] == "tr":
                    ns_tr(u[1], u[2])
                else:
                    ns_m2(u[1], u[2], u[3])
                if u[0] != "tr":
                    big_idx[0] += 1
                    emit_dma_group(big_idx[0] + 1)

            # ---- emission ----
            # rb0: shared only; weight loads woven; first two NS dma groups
            # land late in rb0 so burst 0 can fire early in rb1.
            xt_load(0, 0)
            w1t_load(0)
            xt_load(0, 1)
            w1t_load(1)
            w2t_load(0)
            w2t_load(1)
            for fc in range(FC):
                if fc + 2 < FC:
                    w1t_load(fc + 2)
                    w2t_load(fc + 2)
                fc_step(0, fc)
                if fc == 6:
                    xt_load(1, 0)
                if fc == 9:
                    xt_load(1, 1)
                if fc == 10:
                    emit_dma_group(0)
                if fc == 13:
                    emit_dma_group(1)
            eager_tail(0)
            y0_out(0)
            for rc in range(1, 4):
                for dh in range(2):
                    mm2b(0, rc, dh)

            # rb1-3: interleave NS units between shared steps.
            # Slot keys: fc index 0..15 (burst emitted after that fc_step),
            # 16 = after y0_out, 17+2*(rc-1)+dh = after mm2b(rc, dh).
            per_rb_slots = [
                # rb1: g0 MM1 phase + transposes
                {1: [0], 4: [1], 6: [2], 8: [3], 10: [4], 12: [5], 14: [6, 7]},
                # rb2: g0 MM2 phase + g1 MM1 start
                {1: [8], 4: [9], 6: [10], 8: [11], 10: [12], 12: [13],
                 14: [14], 17: [15]},
                # rb3: g1 MM1 tail + MM2; last burst after the final mm2b so
                # the closing YS DMA drains under NS matmuls
                {1: [16], 2: [17], 4: [18], 5: [19], 7: [20], 12: [21],
                 19: [22], 22: [23]},
            ]
            for rb in range(1, NRB):
                slot_map = {
                    k: [ns_units[i] for i in v]
                    for k, v in per_rb_slots[rb - 1].items()
                }
                for fc in range(FC):
                    fc_step(rb, fc)
                    if rb < NRB - 1:
                        if fc == 6:
                            xt_load(rb + 1, 0)
                        if fc == 9:
                            xt_load(rb + 1, 1)
                    for u in slot_map.get(fc, []):
                        emit_unit(u)
                eager_tail(rb)
                y0_out(rb)
                for u in slot_map.get(16, []):
                    emit_unit(u)
                for rc in range(1, 4):
                    for dh in range(2):
                        mm2b(rb, rc, dh)
                        for u in slot_map.get(17 + 2 * (rc - 1) + dh, []):
                            emit_unit(u)

    nc.compile()
    return nc


def _prepare_inputs(x, W1S, W2S, W1NS, W2NS):
    x = np.ascontiguousarray(x, dtype=np.float32)
    # [FC, 128, DC, 128] per-fc contiguous blocks of W1S.T
    w1t = np.ascontiguousarray(
        np.asarray(W1S.T, dtype=np.float32)
        .reshape(DC, 128, FC, 128)
        .transpose(2, 1, 0, 3)
        .astype(np.float16)
    )
    w2t = np.ascontiguousarray(W2S.T.astype(np.float16))  # [F, D]
    in_maps = []
    for i in range(NCORES):
        # [NRB, 128, DC, RB]: per row-block, partition-major
        xt = np.ascontiguousarray(
            x[i].T.reshape(DC, 128, NRB, RB).transpose(2, 1, 0, 3).astype(np.float16)
        )
        xi = x[:, S + PPC * i : S + PPC * (i + 1), :]  # [B, 8, D]
        # [128, DC, NG, 4, 16] (batch padded 8->16 for 32B-aligned slices)
        xns4 = (
            xi.transpose(2, 1, 0)  # [D, 8, B]
            .reshape(DC, 128, NG, 4, B)
            .transpose(1, 0, 2, 3, 4)
            .astype(np.float16)
        )  # [128, DC, NG, 4, B]
        xns = np.zeros((128, DC, NG, 4, 16), dtype=np.float16)
        xns[..., :B] = xns4
        # W1N [NG, FB, 4, 128, 2*4*512]: [g, fb, dcp, r, half*2048+j*512+c]
        w1n = (
            W1NS[PPC * i : PPC * (i + 1)]
            .astype(np.float16)
            .reshape(NG, 4, 4, 2, 128, FB, 512)  # [g, j, dcp, half, r, fb, c]
            .transpose(0, 5, 2, 4, 3, 1, 6)
            .reshape(NG, FB, 4, 128, 4096)
        )
        # W2N [NG, 2, 8, 128, 2*4*512]: [g, dh, fcp, r, half*2048+j*512+d']
        w2n = (
            W2NS[PPC * i : PPC * (i + 1)]
            .astype(np.float16)
            .reshape(NG, 4, 8, 2, 128, 2, 512)  # [g, j, fcp, half, r, dh, d']
            .transpose(0, 5, 2, 4, 3, 1, 6)
            .reshape(NG, 2, 8, 128, 4096)
        )
        in_maps.append(
            {
                "XT": xt,
                "XNS": np.ascontiguousarray(xns),
                "W1T": w1t,
                "W2T": w2t,
                "W1N": np.ascontiguousarray(w1n),
                "W2N": np.ascontiguousarray(w2n),
                "IDEN": np.eye(128, dtype=np.float16),
            }
        )
    return in_maps


def kernel(x, W1S, W2S, W1NS, W2NS):
    global _cached, LAST_RESULTS
    from concourse.bass_utils import run_bass_kernel_spmd

    if _cached is None:
        _cached = _build()
    nc = _cached
    in_maps = _prepare_inputs(x, W1S, W2S, W1NS, W2NS)
    trace = bool(os.environ.get("MIXEDFFN_TRACE"))
    res = run_bass_kernel_spmd(
        nc, in_maps, core_ids=list(range(NCORES)), trace=trace
    )
    LAST_RESULTS = res
    out = np.empty((B, T, D), dtype=np.float32)
    for i in range(NCORES):
        out[i, :S, :] = res.results[i]["YS"].astype(np.float32)
        yn = res.results[i]["YN"].astype(np.float32)  # [NG, 2, 128, 512]
        # [g, dh, 32j+b, d'] -> out[b, S+8i+4g+j, dh*512+d']
        yn = yn.reshape(NG, 2, 4, 32, 512)[:, :, :, :B, :]
        yn = yn.transpose(3, 0, 2, 1, 4).reshape(B, PPC, D)
        out[:, S + PPC * i : S + PPC * (i + 1), :] = yn
    return out
